# revision 3
# baseline (speedup 1.0000x reference)
"""HeteroGAT (2-layer GAT) Trainium2 kernel — 8 NeuronCores, single launch.

Strategy (v2):
  - Host: add self-loops, degree-sort dst nodes into a slot grid
    (tile t in [0,49), core c in [0,8), partition p in [0,128)), so each
    core owns 6272 slot-nodes. Table row of a node = c*6400 + t*128 + p
    (6400-row blocks leave a spare pad row per core). Per-edge indices
    are in SLOT order, shared by both layers, shipped compact
    ([16, 8*S] int16, 16-wrapped, NOT x8-replicated).
  - Device (one SPMD launch):
      node phase: each core computes h|e_s (bf16 table rows) + e_d
        (kept in SBUF) for ITS OWN slot-nodes only, from a bf16 x chunk;
      AllGather T1 (1.6MB/core -> 13.1MB table);
      L1 edge phase per dst tile: dma_gather rows, w=exp(lrelu(e_s+e_d)),
        out = sum(w*h)/sum(w) + b1, relu, @W2cat -> local T2 rows + e_d2
        (SBUF);
      AllGather T2; L2 edge phase -> out slots (bf16).
  - Host: unpermute slots -> natural node order.

Max-subtraction-free segment softmax: out = sum(w*h)/sum(w) is
mathematically identical to the reference's max-stabilized version
(values are small, no overflow).
"""

import numpy as np
import ml_dtypes
from contextlib import ExitStack

import concourse.bacc as bacc
import concourse.tile as tile
from concourse import mybir
from concourse import bass_utils

NCORES = 8
P = 128
N = 50000
IN = 128
H1, C1 = 2, 32
F1 = H1 * C1          # 64
F2 = 32
NTILES = 49           # dst tiles per core
SLOTS = NTILES * P    # 6272 slots per core
BLK = SLOTS + P       # 6400 table rows per core block (1 spare pad row used)
TROWS = NCORES * BLK  # 51200
PAD_A = SLOTS         # global pad row for pass A (core 0 spare), < 32768
PAD_B_LOCAL = 5 * BLK + SLOTS - 32768   # 38272 - 32768 = 5504
NEG_SLOPE = 0.2
BF = mybir.dt.bfloat16
FP = mybir.dt.float32
I16 = mybir.dt.int16

_cache = {}


def host_prep(edge_index):
    loops = np.arange(N, dtype=np.int64)
    src = np.concatenate([np.asarray(edge_index[0]), loops]).astype(np.int64)
    dst = np.concatenate([np.asarray(edge_index[1]), loops]).astype(np.int64)

    deg = np.bincount(dst, minlength=N)
    order = np.argsort(-deg, kind="stable")
    g = np.arange(N)
    node_tile = np.empty(N, np.int32)
    node_core = np.empty(N, np.int32)
    node_part = np.empty(N, np.int32)
    node_tile[order] = g // 1024
    node_core[order] = (g % 1024) // P
    node_part[order] = g % P
    node_slot = node_tile * P + node_part            # local slot in core
    node_row = node_core * BLK + node_slot           # global table row

    r = node_row[src]
    hi = (r >= 32768).astype(np.int64)
    cntA = np.bincount(dst[hi == 0], minlength=N)
    cntB = np.bincount(dst[hi == 1], minlength=N)
    # per-tile max counts over all nodes in the tile's 1024-group
    gcntA = np.zeros(49 * 1024, np.int64)
    gcntB = np.zeros(49 * 1024, np.int64)
    gidx = node_tile * 1024 + node_core * P + node_part
    gcntA[gidx] = cntA
    gcntB[gidx] = cntB
    CA = np.maximum(1, gcntA.reshape(49, 1024).max(axis=1)).astype(np.int32)
    CB = np.maximum(1, gcntB.reshape(49, 1024).max(axis=1)).astype(np.int32)
    Ct = CA + CB
    offs = np.concatenate([[0], np.cumsum(Ct)]).astype(np.int64)
    S = int(Ct.sum())

    # per-edge column within its (dst, pass) run
    key = dst * 2 + hi
    eorder = np.argsort(key, kind="stable")
    ks = key[eorder]
    cnt = np.bincount(ks, minlength=2 * N)
    j = np.arange(len(ks)) - np.concatenate([[0], np.cumsum(cnt)])[ks]
    ds, hs, rs = dst[eorder], hi[eorder], r[eorder]
    t_e = node_tile[ds]
    col = offs[t_e] + np.where(hs == 0, j, CA[t_e] + j)
    val = np.where(hs == 0, rs, rs - 32768).astype(np.int16)

    IDXCOL = np.zeros((NCORES, P, S), np.int16)
    for t in range(NTILES):  # pass-specific pad defaults
        IDXCOL[:, :, offs[t]:offs[t] + CA[t]] = PAD_A
        IDXCOL[:, :, offs[t] + CA[t]:offs[t + 1]] = PAD_B_LOCAL
    IDXCOL[node_core[ds], node_part[ds], col] = val

    # compact dma_gather layout: per tile-pass block, c-major, 16-wrapped
    IDXC = np.zeros((NCORES, 16, 8 * S), np.int16)
    for t in range(NTILES):
        for c0, c1 in ((offs[t], offs[t] + CA[t]),
                       (offs[t] + CA[t], offs[t + 1])):
            M = IDXCOL[:, :, c0:c1]                          # [8, 128, C]
            flat = M.transpose(0, 2, 1).reshape(NCORES, -1)  # c-major
            IDXC[:, :, 8 * c0:8 * c1] = flat.reshape(
                NCORES, -1, 16).transpose(0, 2, 1)           # [8, 16, 8C]
    return IDXC, CA, CB, offs, S, node_core, node_slot


def build(CA, CB, offs, S):
    nc = bacc.Bacc(num_swdge_queues=4)
    xT = nc.dram_tensor("xT", [P, SLOTS], BF, kind="ExternalInput")
    W1 = nc.dram_tensor("W1", [IN, F1], FP, kind="ExternalInput")
    W2 = nc.dram_tensor("W2", [F1, F2], FP, kind="ExternalInput")
    cat1 = nc.dram_tensor("cat1", [1, 192], FP, kind="ExternalInput")  # asrc|adst|b1
    cat2 = nc.dram_tensor("cat2", [1, 96], FP, kind="ExternalInput")   # asrc2|adst2|b2
    ones = nc.dram_tensor("ones", [1, P], FP, kind="ExternalInput")
    padrow1 = nc.dram_tensor("padrow1", [1, 128], BF, kind="ExternalInput")
    padrow2 = nc.dram_tensor("padrow2", [1, 128], BF, kind="ExternalInput")
    IDX = nc.dram_tensor("IDX", [16, 8 * S], I16, kind="ExternalInput")

    T1loc = nc.dram_tensor("T1loc", [BLK, 128], BF, kind="Internal")
    T1full = nc.dram_tensor("T1full", [TROWS, 128], BF, kind="Internal")
    T2loc = nc.dram_tensor("T2loc", [BLK, 128], BF, kind="Internal")
    T2full = nc.dram_tensor("T2full", [TROWS, 128], BF, kind="Internal")
    OUT = nc.dram_tensor("OUT", [SLOTS, F2], BF, kind="ExternalOutput")

    from concourse.masks import make_identity

    with tile.TileContext(nc) as tc, ExitStack() as es:
        cpool = es.enter_context(tc.tile_pool(name="const", bufs=1))
        ppool = es.enter_context(tc.tile_pool(name="psum", bufs=2, space="PSUM"))
        ppoolB = es.enter_context(tc.tile_pool(name="psumB", bufs=2, space="PSUM"))

        sb_ones = cpool.tile([1, P], FP)
        nc.sync.dma_start(out=sb_ones[:], in_=ones[:])
        sb_cat1 = cpool.tile([1, 192], FP)
        nc.sync.dma_start(out=sb_cat1[:], in_=cat1[:])
        sb_cat2 = cpool.tile([1, 96], FP)
        nc.sync.dma_start(out=sb_cat2[:], in_=cat2[:])
        sb_W1 = cpool.tile([IN, F1], FP)
        nc.sync.dma_start(out=sb_W1[:], in_=W1[:])
        sb_W2 = cpool.tile([F1, F2], FP)
        nc.sync.dma_start(out=sb_W2[:], in_=W2[:])
        sb_pad1 = cpool.tile([1, 128], BF)
        nc.sync.dma_start(out=sb_pad1[:], in_=padrow1[:])
        sb_pad2 = cpool.tile([1, 128], BF)
        nc.sync.dma_start(out=sb_pad2[:], in_=padrow2[:])
        ident = cpool.tile([P, P], FP)
        make_identity(nc, ident[:])

        # replicate cat1/cat2 across partitions: ones.T @ cat
        ps_rep = ppool.tile([P, 192], FP, tag="mm")
        nc.tensor.matmul(out=ps_rep[:], lhsT=sb_ones[:], rhs=sb_cat1[:],
                         start=True, stop=True)
        reps = cpool.tile([P, 192], FP)   # asrc_rep|adst_rep|b1_rep
        nc.vector.tensor_copy(out=reps[:], in_=ps_rep[:])
        ps_rep2 = ppool.tile([P, 96], FP, tag="mm")
        nc.tensor.matmul(out=ps_rep2[:], lhsT=sb_ones[:], rhs=sb_cat2[:],
                         start=True, stop=True)
        reps2 = cpool.tile([P, 96], FP)   # asrc2_rep|adst2_rep|b2_rep
        nc.vector.tensor_copy(out=reps2[:], in_=ps_rep2[:])

        # Wcat = [W1 | W1*asrc summed | W1*adst summed]  [128, 68] -> bf16
        Wcat = cpool.tile([IN, 68], FP)
        nc.vector.tensor_copy(out=Wcat[:, 0:64], in_=sb_W1[:])
        tmp = cpool.tile([IN, F1], FP)
        for k, base in ((0, 64), (1, 66)):
            nc.vector.tensor_tensor(out=tmp[:], in0=sb_W1[:],
                                    in1=reps[:, k * 64:(k + 1) * 64],
                                    op=mybir.AluOpType.mult)
            nc.vector.tensor_reduce(
                out=Wcat[:, base:base + 2],
                in_=tmp[:].rearrange("p (h c) -> p h c", h=2),
                axis=mybir.AxisListType.X, op=mybir.AluOpType.add)
        Wcat_bf = cpool.tile([IN, 68], BF)
        nc.vector.tensor_copy(out=Wcat_bf[:], in_=Wcat[:])
        # W2cat = [W2 | W2@asrc2 | W2@adst2]  [64, 34]
        W2cat = cpool.tile([F1, 34], FP)
        nc.vector.tensor_copy(out=W2cat[:, 0:32], in_=sb_W2[:])
        tmp2 = cpool.tile([F1, F2], FP)
        for k, base in ((0, 32), (1, 33)):
            nc.vector.tensor_tensor(out=tmp2[:], in0=sb_W2[:],
                                    in1=reps2[:F1, k * 32:(k + 1) * 32],
                                    op=mybir.AluOpType.mult)
            nc.vector.tensor_reduce(
                out=W2cat[:, base:base + 1],
                in_=tmp2[:].rearrange("p (h c) -> p h c", h=1),
                axis=mybir.AxisListType.X, op=mybir.AluOpType.add)

        opool = es.enter_context(tc.tile_pool(name="out", bufs=1))
        ed_local = opool.tile([P, NTILES, 2], FP)    # e_d1 for own dst slots
        ed2_local = opool.tile([P, NTILES], FP)      # e_d2 for own dst slots

        # ---- node phase: h|es (table) + ed (SBUF) for own slot-nodes ----
        npool = es.enter_context(tc.tile_pool(name="node", bufs=3))
        NB = 7
        for b in range(NTILES // NB):
            xt = npool.tile([P, NB, P], BF, tag="xt")
            nc.sync.dma_start(out=xt[:], in_=xT[:, b * NB * P:(b + 1) * NB * P])
            stage = npool.tile([P, NB, 128], BF, tag="stage")
            for k in range(NB):
                ps = ppool.tile([P, 68], FP, tag="mm")
                nc.tensor.matmul(out=ps[:], lhsT=xt[:, k, :], rhs=Wcat_bf[:],
                                 start=True, stop=True)
                nc.vector.tensor_copy(out=stage[:, k, 0:66], in_=ps[:, 0:66])
                nc.scalar.copy(out=ed_local[:, b * NB + k, :], in_=ps[:, 66:68])
            nc.sync.dma_start(
                out=T1loc[b * NB * P:(b + 1) * NB * P].rearrange(
                    "(k p) c -> p k c", p=P), in_=stage[:])
        nc.sync.dma_start(out=T1loc[SLOTS:SLOTS + 1, :], in_=sb_pad1[:])

        nc.gpsimd.collective_compute(
            "AllGather", mybir.AluOpType.bypass,
            replica_groups=[list(range(NCORES))],
            ins=[T1loc[:]], outs=[T1full[:]])

        # ---- edge phases ----
        ipool = es.enter_context(tc.tile_pool(name="idx", bufs=1))
        epool = es.enter_context(tc.tile_pool(name="edge", bufs=4))
        spool = es.enter_context(tc.tile_pool(name="small", bufs=3))

        idx_all = ipool.tile([P, 8 * S], I16)
        for k in range(8):
            nc.sync.dma_start(out=idx_all[16 * k:16 * (k + 1), :], in_=IDX[:])

        oT2 = opool.tile([P, NTILES, 33], BF)

        for t in range(NTILES):
            ca, cb = int(CA[t]), int(CB[t])
            C = ca + cb
            o8 = 8 * int(offs[t])
            G = epool.tile([P, C, 128], BF, tag="G")
            nc.gpsimd.dma_gather(
                out_ap=G[:, 0:ca, :], in_ap=T1full[:],
                idxs_ap=idx_all[:, o8:o8 + 8 * ca],
                num_idxs=P * ca, num_idxs_reg=P * ca, elem_size=128,
                single_packet=False, queue_num=t % 4)
            nc.gpsimd.dma_gather(
                out_ap=G[:, ca:C, :], in_ap=T1full[32768:],
                idxs_ap=idx_all[:, o8 + 8 * ca:o8 + 8 * C],
                num_idxs=P * cb, num_idxs_reg=P * cb, elem_size=128,
                single_packet=False, queue_num=t % 4)
            w = spool.tile([P, C, 2], BF, tag="w")
            e = spool.tile([P, C], FP, tag="e")
            den = spool.tile([P, 2], FP, tag="den")
            msg = epool.tile([P, C, F1], BF, tag="msg")
            for h in range(H1):
                nc.scalar.activation(
                    out=e[:], in_=G[:, :, 64 + h],
                    func=mybir.ActivationFunctionType.Identity,
                    bias=ed_local[:, t, h:h + 1])
                nc.vector.scalar_tensor_tensor(
                    out=e[:], in0=e[:], scalar=NEG_SLOPE, in1=e[:],
                    op0=mybir.AluOpType.mult, op1=mybir.AluOpType.max)
                nc.scalar.activation(
                    out=w[:, :, h], in_=e[:],
                    func=mybir.ActivationFunctionType.Exp,
                    accum_out=den[:, h:h + 1])
                nc.vector.tensor_tensor(
                    out=msg[:, :, h * C1:(h + 1) * C1],
                    in0=G[:, :, h * C1:(h + 1) * C1],
                    in1=w[:, :, h:h + 1].to_broadcast([P, C, C1]),
                    op=mybir.AluOpType.mult)
            num = spool.tile([P, F1], FP, tag="num")
            nc.vector.tensor_reduce(
                out=num[:], in_=msg[:].rearrange("p c f -> p f c"),
                axis=mybir.AxisListType.X, op=mybir.AluOpType.add)
            nc.vector.tensor_scalar_add(out=den[:], in0=den[:], scalar1=1e-16)
            rec = spool.tile([P, 2], FP, tag="rec")
            nc.vector.reciprocal(out=rec[:], in_=den[:])
            h2 = spool.tile([P, F1], FP, tag="h2")
            for h in range(H1):
                nc.vector.scalar_tensor_tensor(
                    out=h2[:, h * C1:(h + 1) * C1],
                    in0=num[:, h * C1:(h + 1) * C1], scalar=rec[:, h:h + 1],
                    in1=reps[:, 128 + h * C1:128 + (h + 1) * C1],
                    op0=mybir.AluOpType.mult, op1=mybir.AluOpType.add)
            nc.scalar.activation(out=h2[:], in_=h2[:],
                                 func=mybir.ActivationFunctionType.Relu)
            # L2 prep: hh|e_s2|e_d2 = h2 @ W2cat via transpose
            psT = ppoolB.tile([F1, P], FP, tag="T")
            nc.tensor.transpose(out=psT[:], in_=h2[:], identity=ident[:])
            h2T = spool.tile([F1, P], FP, tag="h2T")
            nc.vector.tensor_copy(out=h2T[:], in_=psT[:])
            ps2 = ppoolB.tile([P, 34], FP, tag="mm2")
            nc.tensor.matmul(out=ps2[:], lhsT=h2T[:], rhs=W2cat[:],
                             start=True, stop=True)
            nc.vector.tensor_copy(out=oT2[:, t, :], in_=ps2[:, 0:33])
            nc.scalar.copy(out=ed2_local[:, t:t + 1], in_=ps2[:, 33:34])

        nc.sync.dma_start(
            out=T2loc[0:SLOTS].rearrange("(t p) c -> p t c", p=P)[:, :, 0:33],
            in_=oT2[:])
        nc.sync.dma_start(out=T2loc[SLOTS:SLOTS + 1, :], in_=sb_pad2[:])

        nc.gpsimd.collective_compute(
            "AllGather", mybir.AluOpType.bypass,
            replica_groups=[list(range(NCORES))],
            ins=[T2loc[:]], outs=[T2full[:]])

        oO = opool.tile([P, NTILES, F2], BF)
        for t in range(NTILES):
            ca, cb = int(CA[t]), int(CB[t])
            C = ca + cb
            o8 = 8 * int(offs[t])
            G = epool.tile([P, C, 128], BF, tag="G")
            nc.gpsimd.dma_gather(
                out_ap=G[:, 0:ca, :], in_ap=T2full[:],
                idxs_ap=idx_all[:, o8:o8 + 8 * ca],
                num_idxs=P * ca, num_idxs_reg=P * ca, elem_size=128,
                single_packet=False, queue_num=t % 4)
            nc.gpsimd.dma_gather(
                out_ap=G[:, ca:C, :], in_ap=T2full[32768:],
                idxs_ap=idx_all[:, o8 + 8 * ca:o8 + 8 * C],
                num_idxs=P * cb, num_idxs_reg=P * cb, elem_size=128,
                single_packet=False, queue_num=t % 4)
            w = spool.tile([P, C, 1], BF, tag="w")
            e = spool.tile([P, C], FP, tag="e")
            den = spool.tile([P, 1], FP, tag="den")
            msg = epool.tile([P, C, F2], BF, tag="msg")
            nc.scalar.activation(
                out=e[:], in_=G[:, :, 32],
                func=mybir.ActivationFunctionType.Identity,
                bias=ed2_local[:, t:t + 1])
            nc.vector.scalar_tensor_tensor(
                out=e[:], in0=e[:], scalar=NEG_SLOPE, in1=e[:],
                op0=mybir.AluOpType.mult, op1=mybir.AluOpType.max)
            nc.scalar.activation(
                out=w[:, :, 0], in_=e[:], func=mybir.ActivationFunctionType.Exp,
                accum_out=den[:])
            nc.vector.tensor_tensor(
                out=msg[:], in0=G[:, :, 0:F2],
                in1=w[:].to_broadcast([P, C, F2]),
                op=mybir.AluOpType.mult)
            num = spool.tile([P, F2], FP, tag="num")
            nc.vector.tensor_reduce(
                out=num[:], in_=msg[:].rearrange("p c f -> p f c"),
                axis=mybir.AxisListType.X, op=mybir.AluOpType.add)
            nc.vector.tensor_scalar_add(out=den[:], in0=den[:], scalar1=1e-16)
            rec = spool.tile([P, 1], FP, tag="rec")
            nc.vector.reciprocal(out=rec[:], in_=den[:])
            nc.vector.scalar_tensor_tensor(
                out=oO[:, t, :], in0=num[:], scalar=rec[:, 0:1],
                in1=reps2[:, 64:96],
                op0=mybir.AluOpType.mult, op1=mybir.AluOpType.add)

        nc.sync.dma_start(
            out=OUT[:].rearrange("(t p) c -> p t c", p=P), in_=oO[:])
    nc.compile()
    return nc


def kernel(x, edge_index, W1, a_src1, a_dst1, b1, W2, a_src2, a_dst2, b2,
           _want_trace=False):
    x = np.asarray(x, np.float32)
    IDXC, CA, CB, offs, S, node_core, node_slot = host_prep(edge_index)

    key = ("prog", tuple(CA.tolist()), tuple(CB.tolist()))
    if key not in _cache:
        _cache[key] = build(CA, CB, offs, S)
    nc = _cache[key]

    # per-core x chunks in slot order, transposed, bf16
    xp = np.zeros((NCORES, SLOTS, IN), np.float32)
    xp[node_core, node_slot] = x
    xTp = np.ascontiguousarray(xp.transpose(0, 2, 1)).astype(ml_dtypes.bfloat16)

    cat1 = np.concatenate([np.asarray(a_src1, np.float32).reshape(-1),
                           np.asarray(a_dst1, np.float32).reshape(-1),
                           np.asarray(b1, np.float32).reshape(-1)])[None]
    cat2 = np.concatenate([np.asarray(a_src2, np.float32).reshape(-1),
                           np.asarray(a_dst2, np.float32).reshape(-1),
                           np.asarray(b2, np.float32).reshape(-1)])[None]
    onesr = np.ones((1, P), np.float32)
    padrow1 = np.zeros((1, 128), ml_dtypes.bfloat16)
    padrow1[0, 64:66] = -1e30
    padrow2 = np.zeros((1, 128), ml_dtypes.bfloat16)
    padrow2[0, 32] = -1e30

    in_maps = [
        dict(xT=xTp[c], W1=np.asarray(W1, np.float32),
             W2=np.asarray(W2, np.float32),
             cat1=cat1, cat2=cat2, ones=onesr,
             padrow1=padrow1, padrow2=padrow2, IDX=IDXC[c])
        for c in range(NCORES)
    ]
    import time as _t
    _t0 = _t.time()
    res = bass_utils.run_bass_kernel_spmd(
        nc, in_maps, core_ids=list(range(NCORES)))
    _t1 = _t.time()
    kernel._times = (_t1 - _t0,)

    allout = np.stack([np.asarray(res.results[c]["OUT"], np.float32)
                       for c in range(NCORES)])
    out = allout[node_core, node_slot]

    kernel._last = res
    return out
